# revision 1
# baseline (speedup 1.0000x reference)
"""Trainium2 kernel for ApplyStickerLayer: out = roll(subimg, (80,80), (2,3)) + base_image.

Structure (guaranteed by the layer): subimg is zero outside the 50x50 sticker
at the origin, base_image is zero inside the destination window, and the roll
never wraps -- so per (b, c) channel image (flat, 50176 elems):

    out[bc, f] = base[bc % 3, f] + sub[bc, f - 18000]     (sub oob -> 0)

HW findings driving this design (measured on this part):
  * SBUF AXI port coverage is king: partition p is wired to one of 16 ports.
    DMAs sourcing from partitions 0..15 get ~2 ports (~50 GB/s); partitions
    32..127 cover ALL 16 ports.  So every store sources from a [128, f] tile
    using rows 32..127.
  * SWDGE descriptors cost ~0.7 us fixed (HBM write round trip), so
    descriptors are fat: one 44.8 KB run per (bc) image column chunk.
  * Cross-partition broadcast is only cheap on TensorE: one matmul per column
    chunk replicates base into all 96 images and adds the shifted sticker:

        psum[128, f] = W.T @ x    W [99, 128] = [identity ; channel selector]
                                  x [99, f]   = [96 sub rows ; 3 base rows]

    (columns 0..31 of W are zero; psum rows 32..127 hold images 0..95).
    Pure-base chunks (f outside [18000, 29200)) use only the 3 selector rows.
  * Inputs are cast f32->bf16 during the load DMA; the matmul accumulates in
    f32.  bf16 rounding (~0.4% rel) is far inside the 2e-2 gate.

DVE drains PSUM to SBUF f32 tiles; SWDGE stores them as 96 fat descriptors
per chunk.  Per core ~19.3 MB written + ~4.9 MB read.
"""

import sys

import numpy as np

if "/opt/trn_rl_repo" not in sys.path:
    sys.path.insert(0, "/opt/trn_rl_repo")

import concourse.bacc as bacc
import concourse.bass as bass
import concourse.mybir as mybir
import concourse.tile as tile
from concourse.bass_utils import run_bass_kernel_spmd

N_CORES = 8
B, C, H, W = 256, 3, 224, 224
BS = B // N_CORES  # 32 batches per core
BC = BS * C  # 96 channel images per core
SH, SW = 80, 80
KH, KW = 50, 50

CHW = H * W  # 50176
IMG = C * CHW  # 150528
SHIFT = SH * W + SW  # 18000: the roll as a flat shift
SUB_LEN = (KH - 1) * W + KW + (W - KW)  # 11200: sub cols that can be nonzero
# shifted-sub support inside a channel image: [SHIFT, SHIFT + SUB_LEN)

K = BC + C  # 99: matmul contraction (96 sub rows + 3 base rows)

_F32 = mybir.dt.float32
_BF16 = mybir.dt.bfloat16

DEFAULT_CFG = {
    "fc": 5600,  # column chunk (also the store descriptor length / 4)
    "mm_f": 512,  # matmul free-dim chunk (<= 512, one PSUM bank)
    "psum_bufs": 8,
    "out_bufs": 4,
    "x_bufs": 4,
    "xb_bufs": 4,
    "act_every": 2,  # every act_every-th PSUM evac goes to ScalarE (ACT)
    "store_split": 6,  # chunks >= this store via SWDGE (its ring is load-free by then)
    "swq": 1,  # num_swdge_queues
}


def build_nc(cfg=None):
    cfg = {**DEFAULT_CFG, **(cfg or {})}
    fc_max = cfg["fc"]
    mm_f = cfg["mm_f"]

    nc = bacc.Bacc(
        "TRN2",
        target_bir_lowering=False,
        num_devices=N_CORES,
        num_swdge_queues=cfg["swq"],
    )
    sub = nc.declare_dram_parameter("subimg", [BS, C, H, W], _F32, isOutput=False)
    base = nc.declare_dram_parameter("base", [C, H, W], _F32, isOutput=False)
    wsel = nc.declare_dram_parameter("wsel", [K, 128], _F32, isOutput=False)
    out = nc.declare_dram_parameter("out", [BS, C, H, W], _F32, isOutput=True)

    chunks = []
    c0 = 0
    while c0 < CHW:
        chunks.append((c0, min(fc_max, CHW - c0)))
        c0 += fc_max

    with tile.TileContext(nc) as tc:
        with (
            tc.tile_pool(name="consts", bufs=1) as cpool,
            tc.tile_pool(name="work", bufs=1) as wpool,
            tc.tile_pool(name="psum", bufs=cfg["psum_bufs"], space=bass.MemorySpace.PSUM) as ppool,
        ):
            # 128-wide weights: full-width LDWEIGHTS is ~2x faster than 96
            t_wk = cpool.tile([K, 128], _BF16, tag="wk")
            nc.gpsimd.dma_start(out=t_wk[:, :], in_=wsel[:, :])
            t_w3 = cpool.tile([C, 128], _BF16, tag="w3")
            nc.gpsimd.dma_start(out=t_w3[:, :], in_=wsel[BC:K, :])

            for ci, (c0, fc) in enumerate(chunks):
                # sub columns contributing to out cols [c0, c0+fc):
                # sub j = f - SHIFT clipped to [0, SUB_LEN)
                s_lo = max(0, c0 - SHIFT)
                s_hi = min(SUB_LEN, c0 + fc - SHIFT)
                has_sub = s_hi > s_lo

                if has_sub:
                    t_x = wpool.tile([K, fc_max], _BF16, tag="x", bufs=cfg["x_bufs"])
                    x_lo = s_lo + SHIFT - c0  # x column where sub j = s_lo lands
                    x_hi = x_lo + (s_hi - s_lo)
                    if x_lo > 0:
                        nc.vector.memset(t_x[0:BC, 0:x_lo], 0.0)
                    if x_hi < fc:
                        nc.vector.memset(t_x[0:BC, x_hi:fc], 0.0)
                    nc.gpsimd.dma_start(
                        out=t_x[0:BC, x_lo:x_hi],
                        in_=bass.AP(sub, s_lo, [[CHW, BC], [1, s_hi - s_lo]]),
                    )
                    nc.gpsimd.dma_start(
                        out=t_x[BC:K, 0:fc],
                        in_=bass.AP(base, c0, [[CHW, C], [1, fc]]),
                    )
                else:
                    t_x = wpool.tile([C, fc_max], _BF16, tag="xb", bufs=cfg["xb_bufs"])
                    nc.gpsimd.dma_start(
                        out=t_x[0:C, 0:fc],
                        in_=bass.AP(base, c0, [[CHW, C], [1, fc]]),
                    )

                t_o = wpool.tile([BC, fc_max], _F32, tag="out", bufs=cfg["out_bufs"])
                for mi, m0 in enumerate(range(0, fc, mm_f)):
                    mf = min(mm_f, fc - m0)
                    t_p = ppool.tile([128, mm_f], _F32, tag="psum")
                    if has_sub:
                        nc.tensor.matmul(
                            t_p[:, 0:mf], t_wk[:, :], t_x[:, m0 : m0 + mf]
                        )
                    else:
                        nc.tensor.matmul(
                            t_p[:, 0:mf], t_w3[:, :], t_x[0:C, m0 : m0 + mf]
                        )
                    # PSUM drain: mostly DVE, a slice to the idle ACT engine
                    if (mi % cfg["act_every"]) == cfg["act_every"] - 1:
                        nc.scalar.copy(t_o[:, m0 : m0 + mf], t_p[0:BC, 0:mf])
                    else:
                        nc.vector.tensor_copy(t_o[:, m0 : m0 + mf], t_p[0:BC, 0:mf])
                store_eng = nc.gpsimd if ci >= cfg["store_split"] else nc.sync
                store_eng.dma_start(
                    out=bass.AP(out, c0, [[CHW, BC], [1, fc]]),
                    in_=t_o[:, 0:fc],
                )
    nc.compile()
    return nc


def _make_wsel():
    w = np.zeros((K, 128), dtype=np.float32)
    for bc in range(BC):
        w[bc, bc] = 1.0  # identity for the shifted sub rows
        w[BC + bc % C, bc] = 1.0  # base channel selector
    return w


def run(inputs, cfg=None, trace=False, **kw):
    sub = np.ascontiguousarray(inputs["subimg"], dtype=np.float32)
    basei = np.ascontiguousarray(inputs["base_image"], dtype=np.float32)
    assert sub.shape == (B, C, H, W) and basei.shape == (1, C, H, W)

    nc = build_nc(cfg)
    w = _make_wsel()
    in_maps = [
        {"subimg": sub[i * BS : (i + 1) * BS], "base": basei[0], "wsel": w}
        for i in range(N_CORES)
    ]
    res = run_bass_kernel_spmd(nc, in_maps, list(range(N_CORES)), trace=trace, **kw)
    full = np.concatenate(
        [res.results[i]["out"] for i in range(N_CORES)], axis=0
    ).astype(np.float32, copy=False)
    return full, res


def kernel(**inputs) -> np.ndarray:
    out, _ = run(inputs)
    return out



# revision 3
# speedup vs baseline: 1.2577x; 1.2577x over previous
"""Trainium2 kernel for ApplyStickerLayer: out = roll(subimg, (80,80), (2,3)) + base_image.

Structure (guaranteed by the layer): subimg is zero outside the 50x50 sticker
at the origin, base_image is zero inside the destination window, and the roll
never wraps -- so per (b, c) channel image (flat, 50176 elems):

    out[bc, f] = base[bc % 3, f] + sub[bc, f - 18000]     (sub oob -> 0)

Only columns [18000, 29200) can receive sub contributions; outside that
window out == base exactly.  Design (v2):

  * Output is stored as bf16 (rounding ~0.4% rel, far inside the 2e-2 gate)
    and upcast to f32 on the host -- halves the dominant HBM write stream
    (19.3 MB -> 9.6 MB per core).
  * Pure-base columns (78% of output) NEVER touch PE/PSUM/DVE: base is held
    bf16 in SBUF striped over 64 partitions {32..63, 96..127} (balanced over
    all 16 SBUF AXI ports, clear of the SWDGE descriptor-ring partitions
    0..31), and stores replicate it across the 32 batches with a stride-0
    broadcast dim in the DMA source AP.  MATMUL cost is cols/cycle
    regardless of contraction depth, so this removes ~76 of 98 matmul
    pieces (~33 us of PE time) vs the all-matmul pipeline.
  * Window columns use one matmul per 512-col piece:
        psum[128, f] = W.T @ x,  W [99, 128] = [identity ; channel selector]
        x [99, f] = [96 sub rows ; 3 base rows]   (psum rows 32..127 = images)
    Inputs are cast f32->bf16 during the SWDGE load; accumulation is f32.
    DVE and ACT alternate draining PSUM to bf16 SBUF tiles.
  * Stores alternate across the two HWDGE rings (nc.sync / nc.scalar);
    loads ride the SWDGE (gpsimd) ring, so no ring ever blocks another.

Per core ~9.6 MB written + ~4.9 MB read => ~40 us HBM floor.
"""

import sys

import numpy as np

if "/opt/trn_rl_repo" not in sys.path:
    sys.path.insert(0, "/opt/trn_rl_repo")

import concourse.bacc as bacc
import concourse.bass as bass
import concourse.mybir as mybir
import concourse.tile as tile
from concourse.bass_utils import run_bass_kernel_spmd

N_CORES = 8
B, C, H, W = 256, 3, 224, 224
BS = B // N_CORES  # 32 batches per core
BC = BS * C  # 96 channel images per core
SH, SW = 80, 80
KH, KW = 50, 50

CHW = H * W  # 50176
IMG = C * CHW  # 150528
SHIFT = SH * W + SW  # 18000: the roll as a flat shift
SUB_LEN = (KH - 1) * W + W  # 11200: sub cols that can be nonzero
W0, W1 = SHIFT, SHIFT + SUB_LEN  # sub window [18000, 29200)

K = BC + C  # 99: matmul contraction (96 sub rows + 3 base rows)
ROW0 = 32  # images live on psum/sbuf rows 32..127

NS, SL = 64, CHW // 64  # 64 stripes x 784 cols = one channel image
HALVES = ((32, 0), (96, 32))  # (partition base, stripe base)

_F32 = mybir.dt.float32
_BF16 = mybir.dt.bfloat16

DEFAULT_CFG = {
    "mm_f": 512,  # matmul free-dim piece (<= 512, one PSUM bank)
    "nb": 2,  # region-B column chunks
    "psum_bufs": 8,
    "out_bufs": 2,
    "x_bufs": 2,
    "act_every": 2,  # every act_every-th PSUM evac goes to ScalarE (ACT)
    "swq": 1,  # num_swdge_queues
}


def _stripe_ops():
    """Striped pure-base store ops: (channel, part0, nstripes, dst_col0, w0, wn).

    Full-stripe runs and the partial stripes at the window edges, per channel.
    dst_col0 is the first out column; w0/wn the within-stripe col range.
    """
    ops = []
    sA_full = W0 // SL  # 22 full stripes before the window
    pA = W0 - sA_full * SL  # 752 cols of stripe 22 before the window
    sC_part = W1 // SL  # stripe 37 straddles the window end
    wC = W1 - sC_part * SL  # window covers first 192 cols of stripe 37
    for c in range(C):
        # region A full stripes 0..21 (half 0)
        ops.append((c, 32, sA_full, 0, 0, SL))
        # region A partial: stripe 22 cols [0, 752)
        ops.append((c, 32 + sA_full, 1, sA_full * SL, 0, pA))
        # region C partial: stripe 37 cols [192, 784)
        ops.append((c, 96 + (sC_part - 32), 1, W1, wC, SL - wC))
        # region C full stripes 38..63 (half 1)
        ops.append((c, 96 + (sC_part - 32) + 1, NS - sC_part - 1, (sC_part + 1) * SL, 0, SL))
    # big ones first on each ring, partials later
    ops.sort(key=lambda o: -(o[2] * o[5]))
    return ops


def build_nc(cfg=None):
    cfg = {**DEFAULT_CFG, **(cfg or {})}
    mm_f = cfg["mm_f"]
    nb = cfg["nb"]
    assert SUB_LEN % nb == 0
    fb = SUB_LEN // nb  # region-B chunk width

    nc = bacc.Bacc(
        "TRN2",
        target_bir_lowering=False,
        num_devices=N_CORES,
        num_swdge_queues=cfg["swq"],
    )
    sub = nc.declare_dram_parameter("subimg", [BS, C, H, W], _F32, isOutput=False)
    base = nc.declare_dram_parameter("base", [C, H, W], _F32, isOutput=False)
    wsel = nc.declare_dram_parameter("wsel", [K, 128], _F32, isOutput=False)
    out = nc.declare_dram_parameter("out", [BS, C, H, W], _BF16, isOutput=True)

    rings = [nc.sync, nc.scalar]

    with tile.TileContext(nc) as tc:
        with (
            tc.tile_pool(name="consts", bufs=1) as cpool,
            tc.tile_pool(name="work", bufs=1) as wpool,
            tc.tile_pool(name="psum", bufs=cfg["psum_bufs"], space=bass.MemorySpace.PSUM) as ppool,
        ):
            # --- loads (SWDGE ring, in gating order) ---
            # striped base: partition pb+i holds stripe sb+i as [c0|c1|c2] runs
            t_rep = cpool.tile([128, C * SL], _BF16, tag="rep")
            for pb, sb in HALVES:
                nc.gpsimd.dma_start(
                    out=t_rep[pb : pb + 32, 0 : C * SL],
                    in_=bass.AP(base, sb * SL, [[SL, 32], [CHW, C], [1, SL]]),
                )
            t_wk = cpool.tile([K, 128], _BF16, tag="wk")
            nc.gpsimd.dma_start(out=t_wk[:, :], in_=wsel[:, :])

            t_xs, t_os = [], []
            for k in range(nb):
                c0 = W0 + k * fb
                t_x = wpool.tile([K, fb], _BF16, tag="x", bufs=cfg["x_bufs"])
                nc.gpsimd.dma_start(
                    out=t_x[0:BC, 0:fb],
                    in_=bass.AP(sub, c0 - SHIFT, [[CHW, BC], [1, fb]]),
                )
                nc.gpsimd.dma_start(
                    out=t_x[BC:K, 0:fb],
                    in_=bass.AP(base, c0, [[CHW, C], [1, fb]]),
                )
                t_xs.append(t_x)

            # --- pure-base striped stores (HWDGE rings, alternating) ---
            for i, (c, p0, ns, d0, w0, wn) in enumerate(_stripe_ops()):
                src = t_rep[p0 : p0 + ns, c * SL + w0 : c * SL + w0 + wn]
                rings[i % 2].dma_start(
                    out=bass.AP(out, c * CHW + d0, [[SL, ns], [IMG, BS], [1, wn]]),
                    in_=src.unsqueeze(1).broadcast_to((ns, BS, wn)),
                )

            # --- window matmul pipeline ---
            pi = 0
            for k in range(nb):
                t_o = wpool.tile([128, fb], _BF16, tag="out", bufs=cfg["out_bufs"])
                for m0 in range(0, fb, mm_f):
                    mf = min(mm_f, fb - m0)
                    t_p = ppool.tile([128, mm_f], _F32, tag="psum")
                    nc.tensor.matmul(t_p[:, 0:mf], t_wk[:, :], t_xs[k][:, m0 : m0 + mf])
                    # engine APs must be partition-quad aligned: split 32:64 / 64:128
                    # and alternate the fat half between DVE and ACT
                    if pi % 2 == 0:
                        nc.vector.tensor_copy(t_o[64:128, m0 : m0 + mf], t_p[64:128, 0:mf])
                        nc.scalar.copy(t_o[32:64, m0 : m0 + mf], t_p[32:64, 0:mf])
                    else:
                        nc.scalar.copy(t_o[64:128, m0 : m0 + mf], t_p[64:128, 0:mf])
                        nc.vector.tensor_copy(t_o[32:64, m0 : m0 + mf], t_p[32:64, 0:mf])
                    pi += 1
                t_os.append(t_o)

            # --- window stores (tail of the HWDGE rings) ---
            for k in range(nb):
                c0 = W0 + k * fb
                rings[k % 2].dma_start(
                    out=bass.AP(out, c0, [[CHW, BC], [1, fb]]),
                    in_=t_os[k][ROW0 : ROW0 + BC, 0:fb],
                )
    nc.compile()
    return nc


def _make_wsel():
    w = np.zeros((K, 128), dtype=np.float32)
    for bc in range(BC):
        w[bc, ROW0 + bc] = 1.0  # identity for the shifted sub rows
        w[BC + bc % C, ROW0 + bc] = 1.0  # base channel selector
    return w


def run(inputs, cfg=None, trace=False, **kw):
    sub = np.ascontiguousarray(inputs["subimg"], dtype=np.float32)
    basei = np.ascontiguousarray(inputs["base_image"], dtype=np.float32)
    assert sub.shape == (B, C, H, W) and basei.shape == (1, C, H, W)

    nc = build_nc(cfg)
    w = _make_wsel()
    in_maps = [
        {"subimg": sub[i * BS : (i + 1) * BS], "base": basei[0], "wsel": w}
        for i in range(N_CORES)
    ]
    res = run_bass_kernel_spmd(nc, in_maps, list(range(N_CORES)), trace=trace, **kw)
    full = np.concatenate(
        [np.asarray(res.results[i]["out"]).astype(np.float32) for i in range(N_CORES)],
        axis=0,
    )
    return full, res


def kernel(**inputs) -> np.ndarray:
    out, _ = run(inputs)
    return out


# revision 4
# speedup vs baseline: 1.3489x; 1.0726x over previous
"""Trainium2 kernel for ApplyStickerLayer: out = roll(subimg, (80,80), (2,3)) + base_image.

Structure (guaranteed by the layer): subimg is zero outside the 50x50 sticker
at the origin, base_image is zero inside the destination window, and the roll
never wraps -- so per (b, c) channel image (flat, 50176 elems):

    out[bc, f] = base[bc % 3, f] + sub[bc, f - 18000]     (sub oob -> 0)

Only columns [18000, 29200) can receive sub contributions; outside that
window out == base exactly.  Design (v3):

  * Output is stored as bf16 (rounding ~0.4% rel, far inside the 2e-2 gate)
    and upcast to f32 on the host -- halves the dominant HBM write stream
    (19.3 MB -> 9.6 MB per core).
  * The channel image (50176 cols) is cut at stripe boundaries (64 stripes
    x 784 cols).  Stripes 0..21 (region A) and 38..63 (region C) are pure
    base: they NEVER touch PE/PSUM/DVE.  Base is held bf16 in SBUF striped
    over 64 partitions {32..63, 96..127} (balanced over all 16 SBUF AXI
    ports, clear of the SWDGE descriptor-ring partitions 0..31) and stored
    with a stride-0 broadcast dim replicating across the 32 batches.
    MATMUL cost is cols/cycle regardless of contraction depth, so this
    removes ~3/4 of the PE time an all-matmul pipeline would need.
  * Stripes 22..37 (region B, cols [17248, 29792) - covers the sub window)
    use one matmul per 512-col piece:
        psum[128, f] = W.T @ x,  W [99, 128] = [identity ; channel selector]
        x [99, f] = [96 sub rows ; 3 base rows]   (psum rows 32..127 = images)
    Inputs are cast f32->bf16 during the SWDGE load; accumulation is f32.
    x border cols outside the sub support are memset to zero (DVE).
  * One role per engine, so no sequencer ever blocks another stage's work:
    SP(sync) ring stores A+B, SWDGE(gpsimd) ring loads then stores C,
    DVE and ACT alternate PSUM drains (quad-split 32:64 / 64:128 copies),
    PE only matmuls.

Per core ~9.6 MB written + ~4.9 MB read => ~40 us HBM floor.
"""

import sys

import numpy as np

if "/opt/trn_rl_repo" not in sys.path:
    sys.path.insert(0, "/opt/trn_rl_repo")

import concourse.bacc as bacc
import concourse.bass as bass
import concourse.mybir as mybir
import concourse.tile as tile
from concourse.bass_utils import run_bass_kernel_spmd

N_CORES = 8
B, C, H, W = 256, 3, 224, 224
BS = B // N_CORES  # 32 batches per core
BC = BS * C  # 96 channel images per core
SH, SW = 80, 80
KH, KW = 50, 50

CHW = H * W  # 50176
IMG = C * CHW  # 150528
SHIFT = SH * W + SW  # 18000: the roll as a flat shift
SUB_LEN = (KH - 1) * W + W  # 11200: sub cols that can be nonzero

K = BC + C  # 99: matmul contraction (96 sub rows + 3 base rows)
ROW0 = 32  # images live on psum/sbuf rows 32..127

NS, SL = 64, CHW // 64  # 64 stripes x 784 cols = one channel image
HALVES = ((32, 0), (96, 32))  # (partition base, stripe base)

S_B0 = SHIFT // SL  # 22: first stripe touching the sub window
S_B1 = (SHIFT + SUB_LEN - 1) // SL + 1  # 38: one past the last
W0, W1 = S_B0 * SL, S_B1 * SL  # matmul region B cols [17248, 29792)

_F32 = mybir.dt.float32
_BF16 = mybir.dt.bfloat16

DEFAULT_CFG = {
    "mm_f": 512,  # matmul free-dim piece (<= 512, one PSUM bank)
    "nb": 2,  # region-B column chunks
    "psum_bufs": 8,
    "out_bufs": 2,
    "x_bufs": 2,
}


def build_nc(cfg=None):
    cfg = {**DEFAULT_CFG, **(cfg or {})}
    mm_f = cfg["mm_f"]
    nb = cfg["nb"]
    assert (W1 - W0) % nb == 0
    fb = (W1 - W0) // nb  # region-B chunk width

    nc = bacc.Bacc(
        "TRN2",
        target_bir_lowering=False,
        num_devices=N_CORES,
        num_swdge_queues=1,
    )
    sub = nc.declare_dram_parameter("subimg", [BS, C, H, W], _F32, isOutput=False)
    base = nc.declare_dram_parameter("base", [C, H, W], _F32, isOutput=False)
    wsel = nc.declare_dram_parameter("wsel", [K, 128], _F32, isOutput=False)
    out = nc.declare_dram_parameter("out", [BS, C, H, W], _BF16, isOutput=True)

    with tile.TileContext(nc) as tc:
        with (
            tc.tile_pool(name="consts", bufs=1) as cpool,
            tc.tile_pool(name="work", bufs=1) as wpool,
            tc.tile_pool(name="psum", bufs=cfg["psum_bufs"], space=bass.MemorySpace.PSUM) as ppool,
        ):
            # --- loads (SWDGE ring, in gating order) ---
            # striped base: partition pb+i holds stripe sb+i as [c0|c1|c2] runs
            t_rep = cpool.tile([128, C * SL], _BF16, tag="rep")
            for pb, sb in HALVES:
                nc.gpsimd.dma_start(
                    out=t_rep[pb : pb + 32, 0 : C * SL],
                    in_=bass.AP(base, sb * SL, [[SL, 32], [CHW, C], [1, SL]]),
                )
            t_wk = cpool.tile([K, 128], _BF16, tag="wk")
            nc.gpsimd.dma_start(out=t_wk[:, :], in_=wsel[:, :])

            t_xs = []
            for k in range(nb):
                c0 = W0 + k * fb
                j_lo, j_hi = max(0, c0 - SHIFT), min(SUB_LEN, c0 + fb - SHIFT)
                x_lo, x_hi = j_lo + SHIFT - c0, j_hi + SHIFT - c0
                t_x = wpool.tile([K, fb], _BF16, tag="x", bufs=cfg["x_bufs"])
                if x_lo > 0:
                    nc.vector.memset(t_x[0:BC, 0:x_lo], 0.0)
                if x_hi < fb:
                    nc.vector.memset(t_x[0:BC, x_hi:fb], 0.0)
                nc.gpsimd.dma_start(
                    out=t_x[0:BC, x_lo:x_hi],
                    in_=bass.AP(sub, j_lo, [[CHW, BC], [1, j_hi - j_lo]]),
                )
                nc.gpsimd.dma_start(
                    out=t_x[BC:K, 0:fb],
                    in_=bass.AP(base, c0, [[CHW, C], [1, fb]]),
                )
                t_xs.append(t_x)

            # --- pure-base striped stores ---
            # region A (stripes 0..21, half 0) on the SP ring - ready earliest
            for c in range(C):
                nc.sync.dma_start(
                    out=bass.AP(out, c * CHW, [[SL, S_B0], [IMG, BS], [1, SL]]),
                    in_=t_rep[32 : 32 + S_B0, c * SL : (c + 1) * SL]
                    .unsqueeze(1)
                    .broadcast_to((S_B0, BS, SL)),
                )
            # region C (stripes 38..63, half 1) on the SWDGE ring (after loads)
            nsc = NS - S_B1
            for c in range(C):
                nc.gpsimd.dma_start(
                    out=bass.AP(out, c * CHW + W1, [[SL, nsc], [IMG, BS], [1, SL]]),
                    in_=t_rep[96 + S_B1 - 32 : 96 + NS - 32, c * SL : (c + 1) * SL]
                    .unsqueeze(1)
                    .broadcast_to((nsc, BS, SL)),
                )

            # --- region B matmul pipeline ---
            pi = 0
            for k in range(nb):
                t_o = wpool.tile([128, fb], _BF16, tag="out", bufs=cfg["out_bufs"])
                for m0 in range(0, fb, mm_f):
                    mf = min(mm_f, fb - m0)
                    t_p = ppool.tile([128, mm_f], _F32, tag="psum")
                    nc.tensor.matmul(t_p[:, 0:mf], t_wk[:, :], t_xs[k][:, m0 : m0 + mf])
                    # engine APs must be partition-quad aligned: 32:64 / 64:128
                    eng = nc.vector.tensor_copy if pi % 2 == 0 else nc.scalar.copy
                    eng(t_o[32:64, m0 : m0 + mf], t_p[32:64, 0:mf])
                    eng(t_o[64:128, m0 : m0 + mf], t_p[64:128, 0:mf])
                    pi += 1
                # window store on the SP ring
                nc.sync.dma_start(
                    out=bass.AP(out, W0 + k * fb, [[CHW, BC], [1, fb]]),
                    in_=t_o[ROW0 : ROW0 + BC, 0:fb],
                )
    nc.compile()
    return nc


def _make_wsel():
    w = np.zeros((K, 128), dtype=np.float32)
    for bc in range(BC):
        w[bc, ROW0 + bc] = 1.0  # identity for the shifted sub rows
        w[BC + bc % C, ROW0 + bc] = 1.0  # base channel selector
    return w


def run(inputs, cfg=None, trace=False, **kw):
    sub = np.ascontiguousarray(inputs["subimg"], dtype=np.float32)
    basei = np.ascontiguousarray(inputs["base_image"], dtype=np.float32)
    assert sub.shape == (B, C, H, W) and basei.shape == (1, C, H, W)

    nc = build_nc(cfg)
    w = _make_wsel()
    in_maps = [
        {"subimg": sub[i * BS : (i + 1) * BS], "base": basei[0], "wsel": w}
        for i in range(N_CORES)
    ]
    res = run_bass_kernel_spmd(nc, in_maps, list(range(N_CORES)), trace=trace, **kw)
    full = np.concatenate(
        [np.asarray(res.results[i]["out"]).astype(np.float32) for i in range(N_CORES)],
        axis=0,
    )
    return full, res


def kernel(**inputs) -> np.ndarray:
    out, _ = run(inputs)
    return out
